# revision 14
# baseline (speedup 1.0000x reference)
"""Trainium2 kernel for nn_AttentionPredictor_33449205301963 (GNN gather).

Math note: in the reference, softmax is taken over an axis of size 1, so the
gate is exactly 1.0 and the whole gate computation cancels:

    out[e] = sum_f h[edge_src[e], f]

i.e. a per-edge row-gather of h followed by a feature-dim sum.

Implementation on 8 NeuronCores, edge-parallel (200k edges per core):
  - Row gathers use the custom SWDGE `dma_gather` ucode instruction
    (InstDMAGatherAnt), which takes int16 row indices. 100000 rows don't fit
    in int16, so edges are bucketed on the host by `node % 4`: the rows
    `n == g (mod 4)` form a 25000-row table (row stride 2048 B), and
    `node >> 2` fits int16.
  - Each core gathers its bucketed edges' rows ([128 f32] each) in chunks,
    reduces each row on the DVE (free-dim sum), and writes per-chunk results.
  - Host glue: bucketing/padding of indices, int16 wrapping into the
    [16, n/16] layout dma_gather expects, and inverse-permuting the per-core
    outputs back to edge order. Pure index bookkeeping; all data movement
    and math happen on device.
"""

import numpy as np

import concourse.bacc as bacc
import concourse.mybir as mybir
from concourse.bass_utils import run_bass_kernel_spmd
from concourse.tile import TileContext

N, F, E = 100000, 128, 1600000
NCORES = 8
P = 128

RES = 4                      # residue classes (node % 4)
RROWS = N // RES             # 25000 rows per residue table (int16-safe)
E_CORE = E // NCORES         # 200000 real edges per core

B = 51200                    # bucket capacity per (core, residue); ~6 sigma
BCOLS = B // 16              # 3200 int16 index columns (wrapped layout)
CHUNKS = [8192] * 6 + [2048]  # gather chunk sizes; sum == B
assert sum(CHUNKS) == B

f32 = mybir.dt.float32
i16 = mybir.dt.int16

TRACE = False
TRACE_CORES = None
LAST_EXEC_NS = {}
LAST_RESULTS = {}

_NC_CACHE = {}


def build_gather():
    nc = bacc.Bacc("TRN2", target_bir_lowering=False, debug=False)
    h_in = nc.dram_tensor("h", [N, F], f32, kind="ExternalInput")
    idx_in = nc.dram_tensor("idx16", [RES, P, BCOLS], i16, kind="ExternalInput")
    out = nc.dram_tensor("out_shard", [RES * B], f32, kind="ExternalOutput")
    # residue view: h4[g, r, f] = h[r*4 + g, f]
    h4 = h_in.rearrange("(r four) f -> four r f", four=RES)
    with TileContext(nc) as tc:
        with (
            tc.tile_pool(name="idx", bufs=2) as ipool,
            tc.tile_pool(name="gat", bufs=3) as gpool,
            tc.tile_pool(name="red", bufs=4) as rpool,
        ):
            for g in range(RES):
                idx_tile = ipool.tile([P, BCOLS], i16, tag="idx")
                nc.sync.dma_start(out=idx_tile[:], in_=idx_in[g])
                pos = 0
                for L in CHUNKS:
                    nb = L // P
                    gat = gpool.tile([P, max(CHUNKS) // P, F], f32, tag="gat")
                    nc.gpsimd.dma_gather(
                        out_ap=gat[:, :nb, :],
                        in_ap=h4[g],
                        idxs_ap=idx_tile[:, pos // 16 : (pos + L) // 16],
                        num_idxs=L,
                        num_idxs_reg=L,
                        elem_size=F,
                        elem_step=RES * F,
                        single_packet=False,
                    )
                    red = rpool.tile([P, max(CHUNKS) // P], f32, tag="red")
                    nc.vector.tensor_reduce(
                        out=red[:, :nb],
                        in_=gat[:, :nb, :],
                        axis=mybir.AxisListType.X,
                        op=mybir.AluOpType.add,
                    )
                    nc.sync.dma_start(
                        out=out[g * B + pos : g * B + pos + L].rearrange(
                            "(p b) -> p b", b=nb
                        ),
                        in_=red[:, :nb],
                    )
                    pos += L
    nc.compile()
    return nc


def _device_pos_map():
    """Flat device-output position for bucket-local index i (fixed layout:
    gathered row i of a chunk of length L lands at [i%128, i//128])."""
    devmap = np.empty(B, dtype=np.int64)
    off = 0
    for L in CHUNKS:
        i = np.arange(L)
        devmap[off : off + L] = off + (i % P) * (L // P) + (i // P)
        off += L
    return devmap


def _run(nc, in_maps, tag):
    kw = {}
    if TRACE:
        kw["trace"] = True
        if TRACE_CORES is not None:
            kw["trace_cores"] = TRACE_CORES
    res = run_bass_kernel_spmd(nc, in_maps, core_ids=list(range(NCORES)), **kw)
    LAST_EXEC_NS[tag] = res.exec_time_ns
    LAST_RESULTS[tag] = res
    return res.results


def kernel(h=None, W=None, b=None, edge_src=None, edge_dst=None, **_unused):
    h = np.ascontiguousarray(np.asarray(h), dtype=np.float32)
    src = np.asarray(edge_src).astype(np.int64)
    assert h.shape == (N, F) and src.shape == (E,)

    devmap = _device_pos_map()
    in_maps = []
    sels = []  # (core, residue) -> original positions within the core slice
    for k in range(NCORES):
        sk = src[k * E_CORE : (k + 1) * E_CORE]
        g = sk & (RES - 1)
        q = (sk >> 2).astype(np.int16)
        arr = np.zeros((RES, P, BCOLS), dtype=np.int16)
        core_sels = []
        for r in range(RES):
            sel = np.flatnonzero(g == r)
            cnt = len(sel)
            assert cnt <= B, f"bucket overflow: {cnt} > {B}"
            tmp = np.zeros(B, dtype=np.int16)
            tmp[:cnt] = q[sel]
            arr[r] = np.tile(tmp.reshape(BCOLS, 16).T, (NCORES, 1))
            core_sels.append(sel)
        sels.append(core_sels)
        in_maps.append({"h": h, "idx16": arr})

    if "gather" not in _NC_CACHE:
        _NC_CACHE["gather"] = build_gather()
    results = _run(_NC_CACHE["gather"], in_maps, "gather")

    out = np.empty(E, dtype=np.float32)
    for k in range(NCORES):
        dev = results[k]["out_shard"]
        ok = out[k * E_CORE : (k + 1) * E_CORE]
        for r in range(RES):
            sel = sels[k][r]
            ok[sel] = dev[r * B + devmap[: len(sel)]]
    return np.ascontiguousarray(out)
